# revision 1
# baseline (speedup 1.0000x reference)
"""BatchTopKSAE Trainium2 kernel.

Strategy (feature-sharded over 8 NeuronCores):
  encode : postT[fc,b] = relu(xT.T @ W_encT + b_enc) via bf16 hi/lo pair
           GEMM (3 matmuls, ~1e-5 relative precision), postT kept fp32.
  top-k  : batch-global threshold t = (k*B)-th largest activation.
           Per (feature,256-batch-chunk) top-8 candidates via DVE max8;
           sampled 128-probe ladder -> 48 exact probe counts (AllReduce)
           -> per-core window extract (max8) -> AllGather (8K window vals)
           -> 128-probe exact pass on broadcast window -> exact t.
  decode : f = postT * (postT >= t) cast bf16; x_hat_partial = f.T @ W_decT
           (bf16 GEMM); ReduceScatter(add) across cores; host concatenates
           the 8 batch shards and adds b_dec.

Self-contained: hardcodes problem shapes; toolchain from /opt/trn_rl_repo.
"""
import sys

sys.path.insert(0, "/opt/trn_rl_repo")

import functools

import ml_dtypes
import numpy as np

import concourse.bacc as bacc
import concourse.bass_isa as bass_isa
import concourse.mybir as mybir
import concourse.tile as tile
from concourse import bass_utils


F32 = mybir.dt.float32
BF16 = mybir.dt.bfloat16
ALU = mybir.AluOpType
ACTF = mybir.ActivationFunctionType

N_CORES = 8
BIG = 1.0e30
NP2 = 40          # stage-2 exact probe count
DCH = 512         # matmul moving chunk (one fp32 PSUM bank)


def _ladder(n=128, lo=0.25, hi=16.0):
    return np.geomspace(lo, hi, n).astype(np.float32)


def build(B, D, F, K_total, debug_outputs=False, host_reduce=False):
    """Build the SPMD program (same program all cores; data differs)."""
    FC = F // N_CORES
    assert B % 512 == 0 and D % 512 == 0 and FC % 128 == 0
    BH = 512                       # batch slice per encode sweep
    NSW = B // BH                  # encode sweeps
    FT = FC // 128                 # feature tiles per core
    DT = D // 128                  # contraction tiles
    CCH = 256                      # candidate chunk length
    NCH = BH // CCH                # chunks per sweep
    SLOTS = FT * NSW * NCH * 8     # candidate slots per partition
    S0 = NCH * 8                   # sweep-0 slots per fc block
    SFC = max(1, FT // 2)          # sampled fc blocks (complete cells)
    SPF = 8                        # full 8-rank cell per sampled fc block
    SN = SFC * SPF                 # sampled slots per partition
    SCALE = SLOTS / SN             # sample -> full scale (cell-unbiased)
    sigma = float(np.sqrt(max(K_total * (SCALE - 1.0), 1.0)))
    margin = 3.0 * sigma + max(200.0, 0.02 * K_total)
    c_hi = (K_total + margin) / SCALE
    c_lo = max((K_total - margin) / SCALE, 0.0)
    DH = D // 2                    # decode d-half
    DDC = min(DCH, DH)             # decode matmul chunk
    GW = N_CORES * 1024            # gathered window size
    Kf = float(K_total)

    nc = bacc.Bacc("TRN2", target_bir_lowering=False, debug=False,
                   num_devices=N_CORES)
    # ---- I/O ----
    xh_d = nc.dram_tensor("xh", [D, B], BF16, kind="ExternalInput")
    xl_d = nc.dram_tensor("xl", [D, B], BF16, kind="ExternalInput")
    weh_d = nc.dram_tensor("weh", [D, FC], BF16, kind="ExternalInput")
    wel_d = nc.dram_tensor("wel", [D, FC], BF16, kind="ExternalInput")
    wd_d = nc.dram_tensor("wd", [FC, D], BF16, kind="ExternalInput")
    be_d = nc.dram_tensor("be", [128, FT], F32, kind="ExternalInput")
    pr1_d = nc.dram_tensor("pr1", [128, 1], F32, kind="ExternalInput")
    prrow_d = nc.dram_tensor("prrow", [1, 128], F32, kind="ExternalInput")
    j2_d = nc.dram_tensor("j2", [1, NP2], F32, kind="ExternalInput")
    j128_d = nc.dram_tensor("j128", [128, 1], F32, kind="ExternalInput")
    j16_d = nc.dram_tensor("j16", [1, 16], F32, kind="ExternalInput")
    if host_reduce:
        out_d = nc.dram_tensor("out", [B, D], F32, kind="ExternalOutput")
    else:
        out_d = nc.dram_tensor("out", [B // N_CORES, D], F32,
                               kind="ExternalOutput")
    if debug_outputs:
        dbg_t = nc.dram_tensor("dbg_t", [1, 1], F32, kind="ExternalOutput")
        dbg_cnt = nc.dram_tensor("dbg_cnt", [1, NP2], F32,
                                 kind="ExternalOutput")
        dbg_win = nc.dram_tensor("dbg_win", [1, 16], F32,
                                 kind="ExternalOutput")
        dbg_cand = nc.dram_tensor("dbg_cand", [128, SLOTS], F32,
                                  kind="ExternalOutput")
        dbg_g1 = nc.dram_tensor("dbg_g1", [1, 128], F32,
                                kind="ExternalOutput")
        dbg_pp = nc.dram_tensor("dbg_pp", [1, 2], F32,
                                kind="ExternalOutput")
        dbg_pr2 = nc.dram_tensor("dbg_pr2", [1, NP2], F32,
                                 kind="ExternalOutput")
        dbg_samp = nc.dram_tensor("dbg_samp", [2, 512], F32,
                                  kind="ExternalOutput")
        dbg_fin = nc.dram_tensor("dbg_fin", [1, 8], F32,
                                 kind="ExternalOutput")
        dbg_c3 = nc.dram_tensor("dbg_c3", [128, 3], F32,
                                kind="ExternalOutput")
        dbg_postT = nc.dram_tensor("dbg_postT", [128, B], F32,
                                   kind="ExternalOutput")
        dbg_ft = nc.dram_tensor("dbg_ft", [128, B], BF16,
                                kind="ExternalOutput")

    rg = [list(range(N_CORES))]

    with tile.TileContext(nc) as tc:
        with tc.tile_pool(name="sb", bufs=1) as sb, \
             tc.tile_pool(name="ps", bufs=3, space="PSUM") as psp, \
             tc.tile_pool(name="dr", bufs=1, space="DRAM") as drp:

            def st(shape, dtype, tag, bufs=1):
                return sb.tile(shape, dtype, tag=tag, bufs=bufs, name=tag)

            # small constants
            be_sb = st([128, FT], F32, "be")
            nc.sync.dma_start(be_sb[:], be_d.ap())
            pr1 = st([128, 1], F32, "pr1")
            nc.sync.dma_start(pr1[:], pr1_d.ap())
            prrow = st([1, 128], F32, "prrow")
            nc.sync.dma_start(prrow[:], prrow_d.ap())
            j2 = st([1, NP2], F32, "j2")
            nc.sync.dma_start(j2[:], j2_d.ap())
            j128 = st([128, 1], F32, "j128")
            nc.sync.dma_start(j128[:], j128_d.ap())
            j16 = st([1, 16], F32, "j16")
            nc.sync.dma_start(j16[:], j16_d.ap())

            postT_dram = drp.tile([FC, B], F32, tag="postT", name="postT")
            cand = st([128, SLOTS], F32, "cand")

            # ============ Phase 1: encode ============
            for h in range(NSW):
                xh_t, xl_t = [], []
                for d in range(DT):
                    th = st([128, BH], BF16, "xz", bufs=2 * DT + 4)
                    nc.sync.dma_start(
                        th[:], xh_d.ap()[d * 128:(d + 1) * 128,
                                         h * BH:(h + 1) * BH])
                    tl = st([128, BH], BF16, "xz", bufs=2 * DT + 4)
                    nc.sync.dma_start(
                        tl[:], xl_d.ap()[d * 128:(d + 1) * 128,
                                         h * BH:(h + 1) * BH])
                    xh_t.append(th)
                    xl_t.append(tl)
                for fc in range(FT):
                    wsh = st([128, DT * 128], BF16, "ws", bufs=3)
                    nc.sync.dma_start(
                        wsh[:].rearrange("p (t q) -> p t q", q=128),
                        weh_d.ap()[:, fc * 128:(fc + 1) * 128].rearrange(
                            "(t p) q -> p t q", p=128))
                    wsl = st([128, DT * 128], BF16, "ws", bufs=3)
                    nc.sync.dma_start(
                        wsl[:].rearrange("p (t q) -> p t q", q=128),
                        wel_d.ap()[:, fc * 128:(fc + 1) * 128].rearrange(
                            "(t p) q -> p t q", p=128))
                    ps = psp.tile([128, BH], F32, tag="ps", name="ps")
                    for d in range(DT):
                        wh = wsh[:, d * 128:(d + 1) * 128]
                        wl = wsl[:, d * 128:(d + 1) * 128]
                        for it, (lhs, rhs_t) in enumerate(
                                ((wh, xh_t[d]), (wh, xl_t[d]),
                                 (wl, xh_t[d]))):
                            for c in range(0, BH, DCH):
                                nc.tensor.matmul(
                                    ps[:, c:c + DCH], lhs,
                                    rhs_t[:, c:c + DCH],
                                    start=(d == 0 and it == 0),
                                    stop=(d == DT - 1 and it == 2))
                    po = st([128, BH], F32, "ev", bufs=2)
                    for c in range(0, BH, DCH):
                        nc.scalar.activation(po[:, c:c + DCH],
                                             ps[:, c:c + DCH], ACTF.Relu,
                                             bias=be_sb[:, fc:fc + 1],
                                             scale=1.0)
                    nc.sync.dma_start(
                        postT_dram[fc * 128:(fc + 1) * 128,
                                   h * BH:(h + 1) * BH], po[:])
                    for ch in range(NCH):
                        base = ((fc * NSW + h) * NCH + ch) * 8
                        nc.vector.max(out=cand[:, base:base + 8],
                                      in_=po[:, ch * CCH:(ch + 1) * CCH])

            # ============ Phase 2: sampled ladder ============
            samp_row = drp.tile([128, SN], F32, tag="samp", name="samp")
            sweep0 = cand[:].rearrange("p (f s) -> p f s",
                                       s=NSW * NCH * 8)[:, :, 0:S0]
            for i in range(SFC):
                nc.sync.dma_start(
                    samp_row[:, i * SPF:(i + 1) * SPF],
                    sweep0[:, 2 * i, 0:SPF])
            samp_bc = st([128, 128 * SN], F32, "mrgbig")
            nc.sync.dma_start(
                samp_bc[:],
                samp_row[:].rearrange("p s -> (p s)").unsqueeze(0)
                .to_broadcast([128, 128 * SN]))
            SW = 128 * SN
            cnt1 = st([128, 1], F32, "cnt1")
            nchk1 = (SW + 2047) // 2048
            cparts1 = []
            for q in range(nchk1):
                lo_, hi_ = q * 2048, min((q + 1) * 2048, SW)
                scr1 = st([128, hi_ - lo_], BF16, "mrgscr")
                cp = st([128, 1], F32, f"cnt1p{q}")
                nc.vector.tensor_scalar(out=scr1[:], in0=samp_bc[:, lo_:hi_],
                                        scalar1=pr1[:], scalar2=0.0,
                                        op0=ALU.is_ge, op1=ALU.add,
                                        accum_out=cp[:])
                cparts1.append(cp)
            nc.vector.tensor_copy(cnt1[:], cparts1[0][:])
            for cp in cparts1[1:]:
                nc.vector.tensor_tensor(out=cnt1[:], in0=cnt1[:],
                                        in1=cp[:], op=ALU.add)
            c1io = drp.tile([1, 128], F32, tag="c1i", name="c1i")
            c1oo = drp.tile([1, 128], F32, tag="c1o", name="c1o")
            nc.sync.dma_start(c1io[:].rearrange("a b -> b a"), cnt1[:])
            nc.gpsimd.collective_compute("AllReduce", ALU.add,
                                         ins=[c1io.opt()],
                                         outs=[c1oo.opt()],
                                         replica_groups=rg)
            g1 = st([1, 128], F32, "g1")
            nc.sync.dma_start(g1[:], c1oo[:])

            # ============ Phase 3: stage-2 probes ============
            fhi = st([1, 128], F32, "fhi")
            nc.vector.tensor_scalar(out=fhi[:], in0=g1[:], scalar1=c_hi,
                                    scalar2=None, op0=ALU.is_ge)
            mh = st([1, 128], F32, "mh")
            nc.vector.tensor_tensor(out=mh[:], in0=prrow[:], in1=fhi[:],
                                    op=ALU.mult)
            p_lo = st([1, 1], F32, "p_lo")
            nc.vector.tensor_reduce(out=p_lo[:], in_=mh[:],
                                    axis=mybir.AxisListType.X, op=ALU.max)
            flo = st([1, 128], F32, "flo")
            nc.vector.tensor_scalar(out=flo[:], in0=g1[:], scalar1=c_lo,
                                    scalar2=None, op0=ALU.is_le)
            ml = st([1, 128], F32, "ml")
            nfl = st([1, 128], F32, "nfl")
            nc.vector.tensor_scalar(out=nfl[:], in0=flo[:], scalar1=-BIG,
                                    scalar2=BIG, op0=ALU.mult, op1=ALU.add)
            nc.vector.tensor_tensor(out=ml[:], in0=prrow[:], in1=flo[:],
                                    op=ALU.mult)
            nc.vector.tensor_tensor(out=ml[:], in0=ml[:], in1=nfl[:],
                                    op=ALU.add)
            p_hi = st([1, 1], F32, "p_hi")
            nc.vector.tensor_reduce(out=p_hi[:], in_=ml[:],
                                    axis=mybir.AxisListType.X, op=ALU.min)
            rng = st([1, 1], F32, "rng")
            nc.vector.tensor_tensor(out=rng[:], in0=p_hi[:], in1=p_lo[:],
                                    op=ALU.subtract)
            probes2 = st([1, NP2], F32, "probes2")
            nc.vector.tensor_scalar(out=probes2[:], in0=j2[:],
                                    scalar1=rng[:], scalar2=p_lo[:],
                                    op0=ALU.mult, op1=ALU.add)
            probes2b = st([128, NP2], F32, "probes2b")
            nc.gpsimd.partition_broadcast(probes2b[:], probes2[:])

            # ============ Phase 4: exact stage-2 counts ============
            scr2 = st([128, min(SLOTS, 2048)], BF16, "mrgscr")
            cnt2 = st([128, NP2], F32, "cnt2")
            for j in range(NP2):
                nc.vector.tensor_scalar(out=scr2[:], in0=cand[:],
                                        scalar1=probes2b[:, j:j + 1],
                                        scalar2=0.0, op0=ALU.is_ge,
                                        op1=ALU.add,
                                        accum_out=cnt2[:, j:j + 1])
            par2 = st([128, NP2], F32, "par2")
            nc.gpsimd.partition_all_reduce(par2[:], cnt2[:], channels=128,
                                           reduce_op=bass_isa.ReduceOp.add)
            c2io = drp.tile([1, NP2], F32, tag="c2i", name="c2i")
            c2oo = drp.tile([1, NP2], F32, tag="c2o", name="c2o")
            nc.sync.dma_start(c2io[:], par2[0:1, :])
            nc.gpsimd.collective_compute("AllReduce", ALU.add,
                                         ins=[c2io.opt()],
                                         outs=[c2oo.opt()],
                                         replica_groups=rg)
            g2 = st([1, NP2], F32, "g2")
            nc.sync.dma_start(g2[:], c2oo[:])

            # ============ Phase 5: window pick + extract ============
            f2 = st([1, NP2], F32, "f2")
            nc.vector.tensor_scalar(out=f2[:], in0=g2[:], scalar1=Kf,
                                    scalar2=None, op0=ALU.is_ge)
            w1 = st([1, NP2], F32, "w1s")
            nc.vector.tensor_tensor(out=w1[:], in0=probes2[:], in1=f2[:],
                                    op=ALU.mult)
            tau_a = st([1, 1], F32, "tau_a")
            nc.vector.tensor_reduce(out=tau_a[:], in_=w1[:],
                                    axis=mybir.AxisListType.X, op=ALU.max)
            w2s = st([1, NP2], F32, "w2s")
            nb2 = st([1, NP2], F32, "nb2")
            nc.vector.tensor_scalar(out=nb2[:], in0=f2[:], scalar1=-BIG,
                                    scalar2=BIG, op0=ALU.mult, op1=ALU.add)
            nc.vector.tensor_tensor(out=w2s[:], in0=g2[:], in1=f2[:],
                                    op=ALU.mult)
            nc.vector.tensor_tensor(out=w2s[:], in0=w2s[:], in1=nb2[:],
                                    op=ALU.add)
            C_a = st([1, 1], F32, "C_a")
            nc.vector.tensor_reduce(out=C_a[:], in_=w2s[:],
                                    axis=mybir.AxisListType.X, op=ALU.min)
            nf2 = st([1, NP2], F32, "nf2")
            nc.vector.tensor_scalar(out=nf2[:], in0=f2[:], scalar1=-1.0,
                                    scalar2=1.0, op0=ALU.mult, op1=ALU.add)
            w3s = st([1, NP2], F32, "w3s")
            bf2 = st([1, NP2], F32, "bf2")
            nc.vector.tensor_scalar(out=bf2[:], in0=f2[:], scalar1=BIG,
                                    scalar2=None, op0=ALU.mult)
            nc.vector.tensor_tensor(out=w3s[:], in0=probes2[:], in1=nf2[:],
                                    op=ALU.mult)
            nc.vector.tensor_tensor(out=w3s[:], in0=w3s[:], in1=bf2[:],
                                    op=ALU.add)
            tau_b = st([1, 1], F32, "tau_b")
            nc.vector.tensor_reduce(out=tau_b[:], in_=w3s[:],
                                    axis=mybir.AxisListType.X, op=ALU.min)
            tab = st([128, 1], F32, "tab")
            nc.gpsimd.partition_broadcast(tab[:], tau_a[:])
            tbb = st([128, 1], F32, "tbb")
            nc.gpsimd.partition_broadcast(tbb[:], tau_b[:])
            # window members or 0 (in place over cand; cand's last use)
            nc.vector.scalar_tensor_tensor(out=cand[:], in0=cand[:],
                                           scalar=tab[:], in1=cand[:],
                                           op0=ALU.is_ge, op1=ALU.mult)
            nc.vector.scalar_tensor_tensor(out=cand[:], in0=cand[:],
                                           scalar=tbb[:], in1=cand[:],
                                           op0=ALU.is_lt, op1=ALU.mult)
            wm8 = st([128, 8], F32, "wm8")
            nc.vector.max(out=wm8[:], in_=cand[:])

            # ============ Phase 6: AllGather window + exact t ============
            win_i = drp.tile([128, 8], F32, tag="win_i", name="win_i")
            win_o = drp.tile([1, GW], F32, tag="win_o", name="win_o")
            nc.sync.dma_start(win_i[:], wm8[:])
            nc.gpsimd.collective_compute("AllGather", ALU.bypass,
                                         ins=[win_i.opt()],
                                         outs=[win_o.opt()],
                                         replica_groups=rg)
            gath = st([128, GW], F32, "mrgbig")
            nc.sync.dma_start(gath[:], win_o[:].to_broadcast([128, GW]))
            rng3 = st([1, 1], F32, "rng3")
            nc.vector.tensor_tensor(out=rng3[:], in0=tau_b[:],
                                    in1=tau_a[:], op=ALU.subtract)
            rng3b = st([128, 1], F32, "rng3b")
            nc.gpsimd.partition_broadcast(rng3b[:], rng3[:])
            probes3 = st([128, 1], F32, "probes3")
            nc.vector.tensor_scalar(out=probes3[:], in0=j128[:],
                                    scalar1=rng3b[:], scalar2=tab[:],
                                    op0=ALU.mult, op1=ALU.add)
            cnt3 = st([128, 1], F32, "cnt3")
            nchk3 = (GW + 2047) // 2048
            cparts3 = []
            for q in range(nchk3):
                lo_, hi_ = q * 2048, min((q + 1) * 2048, GW)
                scr3 = st([128, hi_ - lo_], BF16, "mrgscr")
                cp3 = st([128, 1], F32, f"cnt3p{q}")
                nc.vector.tensor_scalar(out=scr3[:], in0=gath[:, lo_:hi_],
                                        scalar1=probes3[:], scalar2=0.0,
                                        op0=ALU.is_ge, op1=ALU.add,
                                        accum_out=cp3[:])
                cparts3.append(cp3)
            nc.vector.tensor_copy(cnt3[:], cparts3[0][:])
            for cp3 in cparts3[1:]:
                nc.vector.tensor_tensor(out=cnt3[:], in0=cnt3[:],
                                        in1=cp3[:], op=ALU.add)
            wa = st([128, 1], F32, "wa")
            nc.gpsimd.partition_broadcast(wa[:], cnt3[0:1, :])
            cab = st([128, 1], F32, "cab")
            nc.gpsimd.partition_broadcast(cab[:], C_a[:])
            c3g = st([128, 1], F32, "c3g")
            nc.vector.tensor_tensor(out=c3g[:], in0=cnt3[:], in1=wa[:],
                                    op=ALU.subtract)
            nc.vector.tensor_tensor(out=c3g[:], in0=c3g[:], in1=cab[:],
                                    op=ALU.add)
            f3 = st([128, 1], F32, "f3")
            nc.vector.tensor_scalar(out=f3[:], in0=c3g[:], scalar1=Kf,
                                    scalar2=None, op0=ALU.is_ge)
            pf = st([128, 1], F32, "pf")
            nc.vector.tensor_tensor(out=pf[:], in0=probes3[:], in1=f3[:],
                                    op=ALU.mult)
            tlo = st([128, 1], F32, "tlo")
            nc.gpsimd.partition_all_reduce(tlo[:], pf[:], channels=128,
                                           reduce_op=bass_isa.ReduceOp.max)
            nf3 = st([128, 1], F32, "nf3")
            nc.vector.tensor_scalar(out=nf3[:], in0=f3[:], scalar1=-1.0,
                                    scalar2=1.0, op0=ALU.mult, op1=ALU.add)
            cbv = st([128, 1], F32, "cbv")
            nc.vector.tensor_tensor(out=cbv[:], in0=cab[:], in1=wa[:],
                                    op=ALU.subtract)
            # C_hi = C3 at first unflagged probe = max over unflagged C3
            # (C3 monotone decreasing); all-flagged fallback = C_b.
            m1 = st([128, 1], F32, "m1")
            nc.vector.tensor_tensor(out=m1[:], in0=c3g[:], in1=nf3[:],
                                    op=ALU.mult)
            nc.vector.tensor_tensor(out=m1[:], in0=m1[:], in1=cbv[:],
                                    op=ALU.max)
            chi = st([128, 1], F32, "chi")
            nc.gpsimd.partition_all_reduce(chi[:], m1[:], channels=128,
                                           reduce_op=bass_isa.ReduceOp.max)
            p1m = st([128, 1], F32, "p1m")
            nc.vector.tensor_tensor(out=p1m[:], in0=probes3[:], in1=nf3[:],
                                    op=ALU.mult)
            bigf = st([128, 1], F32, "bigf")
            nc.vector.tensor_scalar(out=bigf[:], in0=f3[:], scalar1=BIG,
                                    scalar2=None, op0=ALU.mult)
            nc.vector.tensor_tensor(out=p1m[:], in0=p1m[:], in1=bigf[:],
                                    op=ALU.add)
            nc.vector.tensor_scalar(out=p1m[:], in0=p1m[:], scalar1=-1.0,
                                    scalar2=None, op0=ALU.mult)
            thi_n = st([128, 1], F32, "thi_n")
            nc.gpsimd.partition_all_reduce(thi_n[:], p1m[:], channels=128,
                                           reduce_op=bass_isa.ReduceOp.max)
            thi = st([128, 1], F32, "thi")
            nc.vector.tensor_scalar(out=thi[:], in0=thi_n[:], scalar1=-1.0,
                                    scalar2=None, op0=ALU.mult)
            # bracket members on partition 0 (in place over gath row 0)
            g0 = gath[0:1, :]
            nc.vector.scalar_tensor_tensor(out=g0, in0=g0,
                                           scalar=tlo[0:1, :], in1=g0,
                                           op0=ALU.is_ge, op1=ALU.mult)
            nc.vector.scalar_tensor_tensor(out=g0, in0=g0,
                                           scalar=thi[0:1, :], in1=g0,
                                           op0=ALU.is_lt, op1=ALU.mult)
            z = st([1, 16], F32, "z16")
            nc.vector.max(out=z[:, 0:8], in_=g0)
            nc.vector.match_replace(out=g0, in_to_replace=z[:, 0:8],
                                    in_values=g0, imm_value=0.0)
            nc.vector.max(out=z[:, 8:16], in_=g0)
            rm1 = st([1, 1], F32, "rm1")
            nc.vector.tensor_scalar(out=rm1[:], in0=chi[0:1, :],
                                    scalar1=-1.0, scalar2=Kf - 1.0,
                                    op0=ALU.mult, op1=ALU.add)
            fr = st([1, 16], F32, "fr")
            nc.vector.tensor_scalar(out=fr[:], in0=j16[:], scalar1=rm1[:],
                                    scalar2=None, op0=ALU.is_equal)
            zt = st([1, 16], F32, "zt")
            nc.vector.tensor_tensor(out=zt[:], in0=z[:], in1=fr[:],
                                    op=ALU.mult)
            tval = st([1, 1], F32, "tval")
            nc.vector.tensor_reduce(out=tval[:], in_=zt[:],
                                    axis=mybir.AxisListType.X, op=ALU.add)
            t_bc = st([128, 1], F32, "t_bc")
            nc.gpsimd.partition_broadcast(t_bc[:], tval[:])

            if debug_outputs:
                nc.sync.dma_start(dbg_g1.ap(), g1[:])
                nc.sync.dma_start(dbg_pp.ap()[:, 0:1], p_lo[:])
                nc.sync.dma_start(dbg_pp.ap()[:, 1:2], p_hi[:])
                nc.sync.dma_start(dbg_pr2.ap(), probes2[:])
                nc.sync.dma_start(dbg_samp.ap()[0:1, :],
                                  samp_bc[0:1, 0:512])
                nc.sync.dma_start(dbg_samp.ap()[1:2, :],
                                  samp_bc[5:6, 0:512])
                nc.sync.dma_start(dbg_t.ap(), tval[:])
                nc.sync.dma_start(dbg_fin.ap()[:, 0:1], rm1[:])
                nc.sync.dma_start(dbg_fin.ap()[:, 1:2], chi[0:1, :])
                nc.sync.dma_start(dbg_fin.ap()[:, 2:3], tlo[0:1, :])
                nc.sync.dma_start(dbg_fin.ap()[:, 3:4], thi[0:1, :])
                nc.sync.dma_start(dbg_fin.ap()[:, 4:5], C_a[:])
                nc.sync.dma_start(dbg_fin.ap()[:, 5:6], wa[0:1, :])
                nc.sync.dma_start(dbg_fin.ap()[:, 6:7], tau_a[:])
                nc.sync.dma_start(dbg_fin.ap()[:, 7:8], tau_b[:])
                nc.sync.dma_start(dbg_c3.ap()[:, 0:1], probes3[:])
                nc.sync.dma_start(dbg_c3.ap()[:, 1:2], cnt3[:])
                nc.sync.dma_start(dbg_c3.ap()[:, 2:3], c3g[:])
                nc.sync.dma_start(dbg_cnt.ap(), g2[:])
                nc.sync.dma_start(dbg_win.ap(), z[:])
                nc.sync.dma_start(dbg_cand.ap(), cand[:])

            # ============ Phase 7: mask + decode ============
            ft_t = []
            for fc in range(FT):
                ft = st([128, B], BF16, "ft", bufs=FT)
                for q in range(2):
                    pr = st([128, B // 2], F32, "rld", bufs=3)
                    nc.sync.dma_start(
                        pr[:], postT_dram[fc * 128:(fc + 1) * 128,
                                          q * (B // 2):(q + 1) * (B // 2)])
                    nc.vector.scalar_tensor_tensor(
                        out=ft[:, q * (B // 2):(q + 1) * (B // 2)],
                        in0=pr[:], scalar=t_bc[:], in1=pr[:],
                        op0=ALU.is_ge, op1=ALU.mult)
                ft_t.append(ft)
            if debug_outputs:
                nc.sync.dma_start(dbg_postT.ap(), postT_dram[0:128, :])
                nc.sync.dma_start(dbg_ft.ap(), ft_t[0][:])
            partial = drp.tile([B, D], F32, tag="partial", name="partial")
            for dh in range(2):
                wd_t = []
                for fc in range(FT):
                    wt = st([128, DH], BF16, "wd", bufs=FT)
                    nc.sync.dma_start(
                        wt[:], wd_d.ap()[fc * 128:(fc + 1) * 128,
                                         dh * DH:(dh + 1) * DH])
                    wd_t.append(wt)
                for b in range(B // 128):
                    ps2 = psp.tile([128, DH], F32, tag="ps", name="ps2")
                    for fc in range(FT):
                        for c in range(0, DH, DDC):
                            nc.tensor.matmul(
                                ps2[:, c:c + DDC],
                                ft_t[fc][:, b * 128:(b + 1) * 128],
                                wd_t[fc][:, c:c + DDC],
                                start=(fc == 0), stop=(fc == FT - 1))
                    for c in range(0, DH, DDC):
                        xe = st([128, DDC], F32, "ev", bufs=2)
                        nc.scalar.activation(xe[:], ps2[:, c:c + DDC],
                                             ACTF.Copy)
                        nc.sync.dma_start(
                            partial[b * 128:(b + 1) * 128,
                                    dh * DH + c:dh * DH + c + DDC], xe[:])

            # ============ Phase 8: reduce across cores ============
            if host_reduce:
                nc.sync.dma_start(out_d.ap(), partial[:])
            else:
                NRS = 2
                RB = B // NRS                    # rows per RS chunk
                SH = RB // N_CORES               # shard rows per chunk
                for c in range(NRS):
                    rs_out = drp.tile([SH, D], F32, tag=f"rs_out{c}",
                                      name=f"rs_out{c}")
                    nc.gpsimd.collective_compute(
                        "ReduceScatter", ALU.add,
                        ins=[partial[c * RB:(c + 1) * RB, :]],
                        outs=[rs_out.opt()],
                        replica_groups=rg)
                    nc.sync.dma_start(
                        out_d.ap()[c * SH:(c + 1) * SH, :], rs_out[:])

    nc.compile()
    return nc


@functools.lru_cache(maxsize=2)
def _get_program(B, D, F, K_total, debug_outputs=False, host_reduce=False):
    return build(B, D, F, K_total, debug_outputs, host_reduce)


def _split_bf16(a):
    hi = a.astype(ml_dtypes.bfloat16)
    lo = (a - hi.astype(np.float32)).astype(ml_dtypes.bfloat16)
    return np.ascontiguousarray(hi), np.ascontiguousarray(lo)


def make_inputs(x, W_enc, b_enc, W_dec, b_dec, k):
    B, D = x.shape
    F = W_enc.shape[0]
    FC = F // N_CORES
    FT = FC // 128
    xT = np.ascontiguousarray((np.asarray(x, np.float32)
                               - np.asarray(b_dec, np.float32)[None, :]).T)
    xh, xl = _split_bf16(xT)
    pr1 = _ladder().reshape(128, 1)
    prrow = _ladder().reshape(1, 128)
    j2 = np.linspace(0.0, 1.0, NP2, dtype=np.float32).reshape(1, NP2)
    j128 = (np.arange(128, dtype=np.float32) / 128.0).reshape(128, 1)
    j16 = np.arange(16, dtype=np.float32).reshape(1, 16)
    in_maps = []
    for c in range(N_CORES):
        weT = np.ascontiguousarray(
            np.asarray(W_enc, np.float32)[c * FC:(c + 1) * FC, :].T)
        weh, wel = _split_bf16(weT)
        wdT = np.ascontiguousarray(
            np.asarray(W_dec, np.float32)[:, c * FC:(c + 1) * FC].T)
        wd = wdT.astype(ml_dtypes.bfloat16)
        be = np.ascontiguousarray(
            np.asarray(b_enc, np.float32)[c * FC:(c + 1) * FC]
            .reshape(FT, 128).T)
        in_maps.append({
            "xh": xh, "xl": xl, "weh": weh, "wel": wel, "wd": wd,
            "be": be, "pr1": pr1, "prrow": prrow, "j2": j2,
            "j128": j128, "j16": j16,
        })
    return in_maps


def kernel(x, W_enc, b_enc, W_dec, b_dec, k, _debug=False,
           _host_reduce=False, _trace=False):
    x = np.asarray(x)
    B, D = x.shape
    F = np.asarray(W_enc).shape[0]
    K_total = int(k) * B
    nc = _get_program(B, D, F, K_total, _debug, _host_reduce)
    in_maps = make_inputs(x, W_enc, b_enc, W_dec, b_dec, k)
    res = bass_utils.run_bass_kernel_spmd(
        nc, in_maps, core_ids=list(range(N_CORES)), trace=_trace)
    b_dec32 = np.asarray(b_dec, np.float32)
    if _host_reduce:
        acc = np.zeros((B, D), dtype=np.float64)
        for c in range(N_CORES):
            acc += res.results[c]["out"]
        out = acc.astype(np.float32) + b_dec32[None, :]
    else:
        NRS = 2
        SH = B // NRS // N_CORES
        out = np.empty((B, D), dtype=np.float32)
        for r in range(N_CORES):
            sh = res.results[r]["out"].reshape(NRS, SH, D)
            for c in range(NRS):
                out[c * (B // NRS) + r * SH:
                    c * (B // NRS) + (r + 1) * SH] = sh[c]
        out = out + b_dec32[None, :]
    if _debug or _trace:
        kernel.last_results = res
    return out.astype(np.float32)

